# revision 1
# baseline (speedup 1.0000x reference)
"""DIN attention layer kernel for Trainium2 (8 NeuronCores, data-parallel over batch).

Reference computation (per batch b):
    att = [q, k, q-k, q*k]            # [T, 4M]
    h1  = relu(att @ W1 + b1)         # [T, D]
    h2  = relu(h1 @ W2 + b2)          # [T, D]
    s   = h2 @ w_score + b_score      # [T, 1]
    attn = softmax(s.T + mask * -1e9) # [1, T]
    out = attn @ values               # [1, D]

Key optimizations:
  * Data-parallel: 8 batches per core (B=64 over 8 cores).
  * Algebraic reassociation of the concat matmul:
        att @ W1 = q@(W1a+W1c) + k@(W1b-W1c) + (q*k)@W1d
    The q term is a per-batch row vector folded into the layer-1 bias,
    so the effective contraction is 512 instead of 1024 (mm1 halved).
  * Transposed-activation layout [feature, token]: weights W1/W2 are used
    as-stored for lhsT (no weight transposes); only keys need a transpose,
    done on the PE via identity matmul.
  * bf16 matmuls (fp32 PSUM accumulation); bias/softmax kept in fp32.
  * Softmax without max-subtraction (scores are O(1); masked lanes are
    exp(-1e9) = 0), sum fused into the Exp activation via accum_out.
  * Software-pipelined emission: attn@values for batch b is emitted inside
    batch b+1's block so the PE never waits on the softmax chain.
"""

import numpy as np

P = 128
B = 8          # batches per core
T = 1024       # tokens
M = 256        # key feature dim
D = 1024       # hidden dim
MC = M // P    # key-feature chunks (2)
DC = D // P    # hidden chunks (8)
TC = T // P    # token chunks (8)
NH = 2         # free-dim halves of 512
NEG = -1.0e9

_built = None


def _ns(h):
    return slice(h * 512, (h + 1) * 512)


def _build():
    import os
    stage = int(os.environ.get("DIN_STAGE", "5"))
    setup_n = int(os.environ.get("DIN_SETUP_N", "99"))
    import concourse.bass as bass
    import concourse.bacc as bacc
    import concourse.mybir as mybir
    import concourse.tile as tile
    from concourse.masks import make_identity
    from contextlib import ExitStack

    F32 = mybir.dt.float32
    BF16 = mybir.dt.bfloat16
    AF = mybir.ActivationFunctionType
    OP = mybir.AluOpType

    nc = bacc.Bacc("TRN2")
    q_d = nc.dram_tensor("query", [B, M], F32, kind="ExternalInput").ap()
    k_d = nc.dram_tensor("keys", [B, T, M], F32, kind="ExternalInput").ap()
    v_d = nc.dram_tensor("values", [B, T, D], F32, kind="ExternalInput").ap()
    m_d = nc.dram_tensor("mask", [B, T], F32, kind="ExternalInput").ap()
    w1_d = nc.dram_tensor("W1", [4 * M, D], F32, kind="ExternalInput").ap()
    b1_d = nc.dram_tensor("b1", [D], F32, kind="ExternalInput").ap()
    w2_d = nc.dram_tensor("W2", [D, D], F32, kind="ExternalInput").ap()
    b2_d = nc.dram_tensor("b2", [D], F32, kind="ExternalInput").ap()
    ws_d = nc.dram_tensor("w_score", [D, 1], F32, kind="ExternalInput").ap()
    out_d = nc.dram_tensor("out", [B, D], F32, kind="ExternalOutput").ap()

    with tile.TileContext(nc) as tc, ExitStack() as ctx:
        cons = ctx.enter_context(tc.tile_pool(name="cons", bufs=1))
        kraw = ctx.enter_context(tc.tile_pool(name="kraw", bufs=2))
        xpool = ctx.enter_context(tc.tile_pool(name="xpool", bufs=2))
        h1pool = ctx.enter_context(tc.tile_pool(name="h1p", bufs=1))
        h2pool = ctx.enter_context(tc.tile_pool(name="h2p", bufs=2))
        vpool = ctx.enter_context(tc.tile_pool(name="vp", bufs=1))
        small = ctx.enter_context(tc.tile_pool(name="small", bufs=2))
        dram = ctx.enter_context(tc.tile_pool(name="dram", bufs=2, space="DRAM"))
        psum_mm = ctx.enter_context(tc.tile_pool(name="psmm", bufs=4, space="PSUM"))
        psum_sc = ctx.enter_context(tc.tile_pool(name="pssc", bufs=2, space="PSUM"))
        psum_vec = ctx.enter_context(tc.tile_pool(name="psvec", bufs=2, space="PSUM"))

        # ---- one-time setup -------------------------------------------------
        identity = cons.tile([P, P], F32)
        make_identity(nc, identity)

        # striped per-channel vectors: [p, c] = vec[c*128 + p]
        b2_sb = cons.tile([P, DC], F32)
        ws_f = cons.tile([P, DC], F32)
        ws_sb = cons.tile([P, DC], BF16)
        qt_f = cons.tile([P, MC, B], F32)
        qt_b = cons.tile([P, MC, B], BF16)
        if setup_n >= 2:
            nc.gpsimd.dma_start(b2_sb, b2_d.rearrange("(c p) -> p c", p=P))
            nc.gpsimd.dma_start(ws_f, ws_d.rearrange("(c p) one -> p (c one)", p=P))
            nc.vector.tensor_copy(ws_sb, ws_f)
            for c in range(MC):
                nc.gpsimd.dma_start(
                    qt_f[:, c, :], q_d[:, c * P:(c + 1) * P].rearrange("b p -> p b")
                )
            nc.vector.tensor_copy(qt_b, qt_f)
        else:
            nc.vector.memset(b2_sb, 0.0)
            nc.vector.memset(ws_sb, 0.01)
            nc.vector.memset(qt_f, 0.01)
            nc.vector.memset(qt_b, 0.01)

        # weights in bf16, contraction dim on partitions (as stored);
        # fp32 DMA staging through the shared kraw slots, cast on the DVE
        w2_sb = cons.tile([P, DC, D], BF16)
        if setup_n >= 3:
            for g in range(4):
                wtmp = kraw.tile([P, MC, D], F32, tag="kraw", name=f"w2tmp{g}")
                nc.sync.dma_start(
                    wtmp, w2_d[g * M:(g + 1) * M, :].rearrange("(c p) d -> p c d", p=P)
                )
                nc.vector.tensor_copy(w2_sb[:, 2 * g:2 * g + 2, :], wtmp)
        else:
            nc.vector.memset(w2_sb, 0.01)

        w1qc = cons.tile([P, MC, D], BF16)   # W1a + W1c
        w1bc = cons.tile([P, MC, D], BF16)   # W1b - W1c
        w1d_sb = cons.tile([P, MC, D], BF16)  # W1d
        if setup_n >= 4:
            tmp_a = kraw.tile([P, MC, D], F32, tag="kraw")
            tmp_c = kraw.tile([P, MC, D], F32, tag="kraw")
            nc.sync.dma_start(tmp_a, w1_d[0:M, :].rearrange("(c p) d -> p c d", p=P))
            nc.sync.dma_start(tmp_c, w1_d[2 * M:3 * M, :].rearrange("(c p) d -> p c d", p=P))
            nc.vector.tensor_add(w1qc, tmp_a, tmp_c)
            tmp_b = kraw.tile([P, MC, D], F32, tag="kraw")
            nc.sync.dma_start(tmp_b, w1_d[M:2 * M, :].rearrange("(c p) d -> p c d", p=P))
            nc.vector.tensor_sub(w1bc, tmp_b, tmp_c)
            tmp_d = kraw.tile([P, MC, D], F32, tag="kraw")
            nc.sync.dma_start(tmp_d, w1_d[3 * M:4 * M, :].rearrange("(c p) d -> p c d", p=P))
            nc.vector.tensor_copy(w1d_sb, tmp_d)
        else:
            nc.vector.memset(w1qc, 0.01)
            nc.vector.memset(w1bc, 0.01)
            nc.vector.memset(w1d_sb, 0.01)

        # RT[p, b, j] = R^T[j*128+p, b] + b1[j*128+p], computed directly on
        # the PE (out = w1qc_chunk.T @ qt_chunk gives [d, b]) so no DMA
        # transpose/scatter is ever needed; bias added during the psum drain
        b1_sb = cons.tile([P, DC], F32)
        nc.gpsimd.dma_start(b1_sb, b1_d.rearrange("(c p) -> p c", p=P))
        rt = cons.tile([P, B, DC], F32)
        if setup_n >= 5:
            for j in range(DC):
                rt_ps = psum_vec.tile([P, B], F32, tag="vec", name=f"rt_ps{j}")
                for c in range(MC):
                    nc.tensor.matmul(
                        rt_ps, w1qc[:, c, j * P:(j + 1) * P], qt_b[:, c, :],
                        start=(c == 0), stop=(c == MC - 1),
                    )
                nc.vector.tensor_scalar(
                    rt[:, :, j], rt_ps, b1_sb[:, j:j + 1], None,
                    op0=OP.add,
                )
        else:
            nc.vector.memset(rt, 0.0)

        if stage == 0:
            o1 = small.tile([1, D], F32, tag="osb")
            nc.vector.tensor_copy(o1[:, 0:DC], rt[0:1, 0, :])
            nc.vector.memset(o1[:, DC:], 0.0)
            for b in range(B):
                nc.sync.dma_start(out_d[b:b + 1, :], o1)
            _stage0 = True
        else:
            _stage0 = False

        # ---- per-batch pipeline --------------------------------------------
        # state carried from batch b to block b+1 (deferred attn @ values)
        carry = {}

        def emit_attn_values(b):
            st = carry.pop(b)
            out_ps = [psum_vec.tile([1, 512], F32, tag="vec", name=f"o_ps{h}") for h in range(NH)]
            for h in range(NH):
                for c in range(TC):
                    nc.tensor.matmul(
                        out_ps[h],
                        st["attn_t"][:, c:c + 1],
                        st["vals"][:, c, _ns(h)],
                        start=(c == 0), stop=(c == TC - 1),
                    )
            out_sb = small.tile([1, D], F32, tag="osb")
            for h in range(NH):
                nc.vector.tensor_scalar_mul(out_sb[:, _ns(h)], out_ps[h], st["rec"])
            nc.sync.dma_start(out_d[b:b + 1, :], out_sb)

        for b in range(B if not _stage0 else 0):
            # load keys and transpose on the PE: X[p, c, t] = keys[b, t, c*128+p]
            keys_b = kraw.tile([P, TC, M], F32, tag="kraw")
            nc.sync.dma_start(keys_b, k_d[b].rearrange("(to p) m -> p to m", p=P))
            mask_t = small.tile([1, T], F32, tag="mask")
            nc.sync.dma_start(mask_t, m_d[b:b + 1, :])

            x_t = xpool.tile([P, MC, T], BF16, tag="X")
            for to in range(TC):
                for c in range(MC):
                    tp = psum_mm.tile([P, P], F32, tag="mm")
                    nc.tensor.transpose(tp, keys_b[:, to, c * P:(c + 1) * P], identity)
                    nc.vector.tensor_copy(x_t[:, c, to * P:(to + 1) * P], tp)

            if stage == 10:
                o1 = small.tile([1, D], F32, tag="osb")
                nc.vector.tensor_copy(o1[:, 0:D // 2], x_t[0:1, 0, 0:D].bitcast(F32))
                nc.vector.memset(o1[:, D // 2:], 0.0)
                nc.sync.dma_start(out_d[b:b + 1, :], o1)
                carry[b] = None
                continue

            # deferred attn@values for the previous batch sits here so the PE
            # is never blocked on the (latency-bound) softmax chain
            if b > 0 and stage >= 5:
                emit_attn_values(b - 1)

            vals_f = vpool.tile([P, TC, D], F32, tag="vals_f")
            nc.sync.dma_start(vals_f, v_d[b].rearrange("(to p) d -> p to d", p=P))
            vals = vpool.tile([P, TC, D], BF16, tag="vals")
            nc.vector.tensor_copy(vals, vals_f)

            # QK[p, c, t] = q[b, c*128+p] * X[p, c, t]
            qk = xpool.tile([P, MC, T], BF16, tag="QK")
            for c in range(MC):
                nc.vector.tensor_scalar_mul(qk[:, c, :], x_t[:, c, :], qt_f[:, c, b:b + 1])

            if stage == 1:
                o1 = small.tile([1, D], F32, tag="osb")
                nc.vector.tensor_copy(o1[:, 0:D // 2], x_t[0:1, 0, 0:D].bitcast(F32))
                nc.vector.memset(o1[:, D // 2:], 0.0)
                nc.sync.dma_start(out_d[b:b + 1, :], o1)
                carry[b] = None
                continue

            # mm1: H1[d, t] = relu(W1bc.T @ X + W1d.T @ QK + bias1)
            h1 = h1pool.tile([P, DC, T], BF16, tag="H1")
            for j in range(DC):
                for h in range(NH):
                    ps = psum_mm.tile([P, 512], F32, tag="mm")
                    for c in range(MC):
                        nc.tensor.matmul(
                            ps, w1bc[:, c, j * P:(j + 1) * P], x_t[:, c, _ns(h)],
                            start=(c == 0), stop=False,
                        )
                    for c in range(MC):
                        nc.tensor.matmul(
                            ps, w1d_sb[:, c, j * P:(j + 1) * P], qk[:, c, _ns(h)],
                            start=False, stop=(c == MC - 1),
                        )
                    nc.scalar.activation(
                        h1[:, j, _ns(h)], ps, AF.Relu, bias=rt[:, b, j:j + 1], scale=1.0
                    )

            if stage == 2:
                o1 = small.tile([1, D], F32, tag="osb")
                nc.vector.tensor_copy(o1[:, 0:D // 2], h1[0:1, 0, 0:D].bitcast(F32))
                nc.vector.memset(o1[:, D // 2:], 0.0)
                nc.sync.dma_start(out_d[b:b + 1, :], o1)
                carry[b] = None
                continue

            # mm2 + score: H2 chunks are consumed immediately by the score
            # matmuls (deferred by one j so the PE doesn't wait on the relu)
            score_ps = [psum_sc.tile([1, 512], F32, tag="sc", name=f"s_ps{h}") for h in range(NH)]
            h2_tiles = {}
            for j in range(DC):
                h2 = h2pool.tile([P, T], BF16, tag="H2")
                h2_tiles[j] = h2
                for h in range(NH):
                    ps = psum_mm.tile([P, 512], F32, tag="mm")
                    for c in range(DC):
                        nc.tensor.matmul(
                            ps, w2_sb[:, c, j * P:(j + 1) * P], h1[:, c, _ns(h)],
                            start=(c == 0), stop=(c == DC - 1),
                        )
                    nc.scalar.activation(
                        h2[:, _ns(h)], ps, AF.Relu, bias=b2_sb[:, j:j + 1], scale=1.0
                    )
                if j > 0:
                    jj = j - 1
                    h2_prev = h2_tiles.pop(jj)
                    for h in range(NH):
                        nc.tensor.matmul(
                            score_ps[h], ws_sb[:, jj:jj + 1], h2_prev[:, _ns(h)],
                            start=(jj == 0), stop=False, skip_group_check=True,
                        )
            jj = DC - 1
            h2_prev = h2_tiles.pop(jj)
            for h in range(NH):
                nc.tensor.matmul(
                    score_ps[h], ws_sb[:, jj:jj + 1], h2_prev[:, _ns(h)],
                    start=False, stop=True, skip_group_check=True,
                )

            if stage == 3:
                o1 = small.tile([1, D], F32, tag="osb")
                for h in range(NH):
                    nc.vector.tensor_copy(o1[:, _ns(h)], score_ps[h])
                nc.sync.dma_start(out_d[b:b + 1, :], o1)
                carry[b] = None
                continue

            # softmax (no max subtraction: scores are O(1), masked lanes
            # underflow to exactly 0). score = mask * -1e9 + raw_score
            score_sb = small.tile([1, T], F32, tag="ssb")
            for h in range(NH):
                nc.vector.scalar_tensor_tensor(
                    score_sb[:, _ns(h)], in0=mask_t[:, _ns(h)], scalar=NEG,
                    in1=score_ps[h], op0=OP.mult, op1=OP.add,
                )
            sum_sb = small.tile([1, 1], F32, tag="sum")
            exp_bf = small.tile([1, T], BF16, tag="expb")
            nc.scalar.activation(exp_bf, score_sb, AF.Exp, accum_out=sum_sb)
            rec = small.tile([1, 1], F32, tag="rec")
            nc.vector.reciprocal(rec, sum_sb)
            # attn_t[p, c] = exp_score[c*128 + p]  (partition-major for lhsT),
            # via a DRAM bounce to keep the SBUF write partition-outer
            attn_dram = dram.tile([1, T], BF16, tag="attn_dram")
            nc.sync.dma_start(attn_dram, exp_bf)
            attn_t = small.tile([P, TC], BF16, tag="attn")
            nc.sync.dma_start(
                attn_t, attn_dram.rearrange("one (c p) -> p (one c)", p=P)
            )
            if stage == 4:
                o1 = small.tile([1, D], F32, tag="osb")
                nc.vector.tensor_copy(o1[:, 0:TC], attn_t[0:1, :])
                nc.vector.memset(o1[:, TC:], 0.0)
                nc.sync.dma_start(out_d[b:b + 1, :], o1)
                carry[b] = None
                continue

            carry[b] = {"attn_t": attn_t, "vals": vals, "rec": rec}

        if stage >= 5 and not _stage0:
            emit_attn_values(B - 1)

    nc.compile()
    return nc


def _get_built():
    global _built
    if _built is None:
        _built = _build()
    return _built


N_CORES = 8


def make_in_maps(query, keys, values, mask, W1, b1, W2, b2, w_score, b_score=None):
    query = np.ascontiguousarray(np.asarray(query, dtype=np.float32).reshape(64, M))
    keys = np.ascontiguousarray(np.asarray(keys, dtype=np.float32))
    values = np.ascontiguousarray(np.asarray(values, dtype=np.float32))
    mask = np.ascontiguousarray(np.asarray(mask, dtype=np.float32).reshape(64, T))
    shared = {
        "W1": np.ascontiguousarray(np.asarray(W1, dtype=np.float32)),
        "b1": np.ascontiguousarray(np.asarray(b1, dtype=np.float32)),
        "W2": np.ascontiguousarray(np.asarray(W2, dtype=np.float32)),
        "b2": np.ascontiguousarray(np.asarray(b2, dtype=np.float32)),
        "w_score": np.ascontiguousarray(np.asarray(w_score, dtype=np.float32)),
    }
    in_maps = []
    for c in range(N_CORES):
        sl = slice(c * B, (c + 1) * B)
        in_maps.append({
            "query": query[sl],
            "keys": keys[sl],
            "values": values[sl],
            "mask": mask[sl],
            **shared,
        })
    return in_maps


def gather_out(results):
    out = np.concatenate([results[c]["out"] for c in range(N_CORES)], axis=0)
    return out.reshape(64, 1, D).astype(np.float32)


def kernel(query, keys, values, mask, W1, b1, W2, b2, w_score, b_score):
    """Full-input entry point: shards over 8 NeuronCores, returns [64, 1, D]."""
    from concourse.bass_utils import run_bass_kernel_spmd

    nc = _get_built()
    in_maps = make_in_maps(query, keys, values, mask, W1, b1, W2, b2, w_score)
    res = run_bass_kernel_spmd(nc, in_maps, core_ids=list(range(N_CORES)))
    return gather_out(res.results)



# revision 2
# speedup vs baseline: 1.9410x; 1.9410x over previous
"""DIN attention layer kernel for Trainium2 (8 NeuronCores, data-parallel over batch).

Reference computation (per batch b):
    att = [q, k, q-k, q*k]            # [T, 4M]
    h1  = relu(att @ W1 + b1)         # [T, D]
    h2  = relu(h1 @ W2 + b2)          # [T, D]
    s   = h2 @ w_score + b_score      # [T, 1]
    attn = softmax(s.T + mask * -1e9) # [1, T]
    out = attn @ values               # [1, D]

Key optimizations:
  * Data-parallel: 8 batches per core (B=64 over 8 cores).
  * Masked-token compaction on the host: tokens with mask=1 get attention
    weight exactly 0 (exp(-1e9) underflows to 0 in fp32, identical to the
    reference's softmax), so they are dropped before the device kernel.
    All matmuls then run on TP <= 1024 tokens (TP = max kept count over
    the 64 batches, rounded up to a multiple of 128; dense fallback at
    TP=1024 keeps the kernel exact for any mask density).
  * Algebraic reassociation of the concat matmul:
        att @ W1 = q@(W1a+W1c) + k@(W1b-W1c) + (q*k)@W1d
    The q term is a per-batch row vector folded into the layer-1 bias;
    the q*k term is folded into the weights per batch:
        (q*k) @ W1d = k @ (diag(q) W1d)
    so layer 1 contracts over just M=256 (vs 1024 naively).
  * Host-side layout prep: keys pre-transposed to [M, TP] bf16 (no PE
    transposes), values bf16, W1 recombined + bf16, W2 bf16.
  * bf16 matmuls (fp32 PSUM accumulation); bias/softmax kept in fp32.
  * Softmax without max-subtraction (scores are O(1); masked/pad lanes are
    exp(-1e9) = 0), sum fused into the Exp activation via accum_out.
  * Layer-1 PSUM drains run on the DVE (tensor_scalar add-bias + max(0))
    for even j, ACT for odd j, so neither engine gates the PE.
  * Software-pipelined emission: attn@values for batch b is emitted inside
    batch b+1's mm2 loop so the PE never waits on the softmax chain.
"""

import numpy as np
import ml_dtypes

P = 128
B = 8          # batches per core
T = 1024       # tokens (full)
M = 256        # key feature dim
D = 1024       # hidden dim
MC = M // P    # key-feature chunks (2)
DC = D // P    # hidden chunks (8)
NEG = -1.0e9

BF16NP = ml_dtypes.bfloat16

_built = {}


def _chunks(tp):
    """Token free-dim chunks of width <= 512."""
    out = []
    off = 0
    while off < tp:
        w = min(512, tp - off)
        out.append((off, w))
        off += w
    return out


def _build(tp):
    import concourse.bass as bass  # noqa: F401 (registers passes)
    import concourse.bacc as bacc
    import concourse.mybir as mybir
    import concourse.tile as tile
    from contextlib import ExitStack

    F32 = mybir.dt.float32
    BF16 = mybir.dt.bfloat16
    AF = mybir.ActivationFunctionType
    OP = mybir.AluOpType

    tc_n = tp // P          # token chunks of 128 (for attn_t / vals layout)
    cks = _chunks(tp)       # free-dim chunks of <=512

    nc = bacc.Bacc("TRN2")
    q_d = nc.dram_tensor("query", [B, M], F32, kind="ExternalInput").ap()
    kt_d = nc.dram_tensor("keys_t", [B, M, tp], BF16, kind="ExternalInput").ap()
    v_d = nc.dram_tensor("values", [B, tp, D], BF16, kind="ExternalInput").ap()
    m_d = nc.dram_tensor("mask", [B, tp], F32, kind="ExternalInput").ap()
    w1qc_d = nc.dram_tensor("w1qc", [M, D], BF16, kind="ExternalInput").ap()
    w1bc_d = nc.dram_tensor("w1bc", [M, D], BF16, kind="ExternalInput").ap()
    w1d_d = nc.dram_tensor("w1d", [M, D], BF16, kind="ExternalInput").ap()
    w2_d = nc.dram_tensor("w2", [D, D], BF16, kind="ExternalInput").ap()
    b1_d = nc.dram_tensor("b1", [D], F32, kind="ExternalInput").ap()
    b2_d = nc.dram_tensor("b2", [D], F32, kind="ExternalInput").ap()
    ws_d = nc.dram_tensor("w_score", [D, 1], F32, kind="ExternalInput").ap()
    out_d = nc.dram_tensor("out", [B, D], F32, kind="ExternalOutput").ap()

    with tile.TileContext(nc) as tc, ExitStack() as ctx:
        cons = ctx.enter_context(tc.tile_pool(name="cons", bufs=1))
        xpool = ctx.enter_context(tc.tile_pool(name="xpool", bufs=2))
        wepool = ctx.enter_context(tc.tile_pool(name="wep", bufs=2))
        h1pool = ctx.enter_context(tc.tile_pool(name="h1p", bufs=1))
        h2pool = ctx.enter_context(tc.tile_pool(name="h2p", bufs=2))
        vpool = ctx.enter_context(tc.tile_pool(name="vp", bufs=2))
        small = ctx.enter_context(tc.tile_pool(name="small", bufs=2))
        dram = ctx.enter_context(tc.tile_pool(name="dram", bufs=2, space="DRAM"))
        psum_mm = ctx.enter_context(tc.tile_pool(name="psmm", bufs=4, space="PSUM"))
        psum_sc = ctx.enter_context(tc.tile_pool(name="pssc", bufs=2, space="PSUM"))
        psum_vec = ctx.enter_context(tc.tile_pool(name="psvec", bufs=2, space="PSUM"))

        # ---- one-time setup -------------------------------------------------
        # striped per-channel vectors: [p, c] = vec[c*128 + p]
        qt_f = cons.tile([P, MC, B], F32)
        qt_b = cons.tile([P, MC, B], BF16)
        for c in range(MC):
            nc.gpsimd.dma_start(
                qt_f[:, c, :], q_d[:, c * P:(c + 1) * P].rearrange("b p -> p b")
            )
        nc.vector.tensor_copy(qt_b, qt_f)
        b1_sb = cons.tile([P, DC], F32)
        nc.gpsimd.dma_start(b1_sb, b1_d.rearrange("(c p) -> p c", p=P))

        # weights, contraction dim on partitions, pre-cast bf16 on the host
        w1qc = cons.tile([P, MC, D], BF16)
        w1bc = cons.tile([P, MC, D], BF16)
        w1d_sb = cons.tile([P, MC, D], BF16)
        nc.sync.dma_start(w1qc, w1qc_d.rearrange("(c p) d -> p c d", p=P))
        nc.sync.dma_start(w1bc, w1bc_d.rearrange("(c p) d -> p c d", p=P))
        nc.sync.dma_start(w1d_sb, w1d_d.rearrange("(c p) d -> p c d", p=P))
        w2_sb = cons.tile([P, DC, D], BF16)
        nc.sync.dma_start(w2_sb, w2_d.rearrange("(c p) d -> p c d", p=P))

        b2_sb = cons.tile([P, DC], F32)
        ws_f = cons.tile([P, DC], F32)
        ws_sb = cons.tile([P, DC], BF16)
        nc.gpsimd.dma_start(b2_sb, b2_d.rearrange("(c p) -> p c", p=P))
        nc.gpsimd.dma_start(ws_f, ws_d.rearrange("(c p) one -> p (c one)", p=P))
        nc.vector.tensor_copy(ws_sb, ws_f)

        # RT[p, b, j] = (q_b @ (W1a+W1c))[j*128+p] + b1[j*128+p], on the PE
        rt = cons.tile([P, B, DC], F32)
        for j in range(DC):
            rt_ps = psum_vec.tile([P, B], F32, tag="vec", name=f"rt_ps{j}")
            for c in range(MC):
                nc.tensor.matmul(
                    rt_ps, w1qc[:, c, j * P:(j + 1) * P], qt_b[:, c, :],
                    start=(c == 0), stop=(c == MC - 1),
                )
            nc.vector.tensor_scalar(
                rt[:, :, j], rt_ps, b1_sb[:, j:j + 1], None, op0=OP.add,
            )

        # ---- per-batch pipeline --------------------------------------------
        carry = {}

        def emit_attn_values(b):
            st = carry.pop(b)
            out_ps = [psum_vec.tile([1, 512], F32, tag="vec", name=f"o_ps{h}")
                      for h in range(2)]
            for h in range(2):
                for c in range(tc_n):
                    nc.tensor.matmul(
                        out_ps[h],
                        st["attn_t"][:, c:c + 1],
                        st["vals"][:, c, h * 512:(h + 1) * 512],
                        start=(c == 0), stop=(c == tc_n - 1),
                    )
            out_sb = small.tile([1, D], F32, tag="osb")
            for h in range(2):
                nc.vector.tensor_scalar_mul(
                    out_sb[:, h * 512:(h + 1) * 512], out_ps[h], st["rec"])
            nc.sync.dma_start(out_d[b:b + 1, :], out_sb)

        for b in range(B):
            # per-batch effective layer-1 weights: W1eff = W1bc + q_b * W1d
            w1eff = wepool.tile([P, MC, D], BF16, tag="weff")
            for c in range(MC):
                nc.vector.scalar_tensor_tensor(
                    w1eff[:, c, :], in0=w1d_sb[:, c, :], scalar=qt_f[:, c, b:b + 1],
                    in1=w1bc[:, c, :], op0=OP.mult, op1=OP.add,
                )

            x_t = xpool.tile([P, MC, tp], BF16, tag="X")
            nc.sync.dma_start(x_t, kt_d[b].rearrange("(c p) t -> p c t", p=P))
            mask_t = small.tile([1, tp], F32, tag="mask")
            nc.sync.dma_start(mask_t, m_d[b:b + 1, :])
            vals = vpool.tile([P, tc_n, D], BF16, tag="vals")
            nc.sync.dma_start(vals, v_d[b].rearrange("(to p) d -> p to d", p=P))

            # mm1: H1[d, t] = relu(W1eff.T @ X + rt_b)
            h1 = h1pool.tile([P, DC, tp], BF16, tag="H1")
            for j in range(DC):
                for off, w in cks:
                    ps = psum_mm.tile([P, w], F32, tag="mm", name=f"m1_{j}_{off}")
                    for c in range(MC):
                        nc.tensor.matmul(
                            ps, w1eff[:, c, j * P:(j + 1) * P],
                            x_t[:, c, off:off + w],
                            start=(c == 0), stop=(c == MC - 1),
                        )
                    if j % 2 == 0:
                        nc.vector.tensor_scalar(
                            h1[:, j, off:off + w], ps, rt[:, b, j:j + 1], 0.0,
                            op0=OP.add, op1=OP.max,
                        )
                    else:
                        nc.scalar.activation(
                            h1[:, j, off:off + w], ps, AF.Relu,
                            bias=rt[:, b, j:j + 1], scale=1.0,
                        )

            # mm2 + score: H2 chunks consumed immediately by the score
            # matmuls (deferred by one j so the PE doesn't wait on the relu)
            score_ps = [psum_sc.tile([1, w], F32, tag="sc", name=f"s_ps{off}")
                        for off, w in cks]
            h2_tiles = {}
            for j in range(DC):
                h2 = h2pool.tile([P, tp], BF16, tag="H2")
                h2_tiles[j] = h2
                for off, w in cks:
                    ps = psum_mm.tile([P, w], F32, tag="mm", name=f"m2_{j}_{off}")
                    for c in range(DC):
                        nc.tensor.matmul(
                            ps, w2_sb[:, c, j * P:(j + 1) * P],
                            h1[:, c, off:off + w],
                            start=(c == 0), stop=(c == DC - 1),
                        )
                    nc.scalar.activation(
                        h2[:, off:off + w], ps, AF.Relu,
                        bias=b2_sb[:, j:j + 1], scale=1.0,
                    )
                if j > 0:
                    jj = j - 1
                    h2_prev = h2_tiles.pop(jj)
                    for ci, (off, w) in enumerate(cks):
                        nc.tensor.matmul(
                            score_ps[ci], ws_sb[:, jj:jj + 1],
                            h2_prev[:, off:off + w],
                            start=(jj == 0), stop=False, skip_group_check=True,
                        )
                # deferred attn@values for the previous batch: emitted behind
                # mm1 + one mm2 j-round of PE work so its softmax/bounce
                # chain is fully hidden
                if j == 1 and b > 0:
                    emit_attn_values(b - 1)
            jj = DC - 1
            h2_prev = h2_tiles.pop(jj)
            for ci, (off, w) in enumerate(cks):
                nc.tensor.matmul(
                    score_ps[ci], ws_sb[:, jj:jj + 1], h2_prev[:, off:off + w],
                    start=False, stop=True, skip_group_check=True,
                )

            # softmax (no max subtraction: scores are O(1), masked/pad lanes
            # underflow to exactly 0). score = mask * -1e9 + raw_score
            score_sb = small.tile([1, tp], F32, tag="ssb")
            for ci, (off, w) in enumerate(cks):
                nc.vector.scalar_tensor_tensor(
                    score_sb[:, off:off + w], in0=mask_t[:, off:off + w],
                    scalar=NEG, in1=score_ps[ci], op0=OP.mult, op1=OP.add,
                )
            sum_sb = small.tile([1, 1], F32, tag="sum")
            exp_bf = small.tile([1, tp], BF16, tag="expb")
            nc.scalar.activation(exp_bf, score_sb, AF.Exp, accum_out=sum_sb)
            rec = small.tile([1, 1], F32, tag="rec")
            nc.vector.reciprocal(rec, sum_sb)
            # attn_t[p, c] = exp_score[c*128 + p]  (partition-major for lhsT),
            # via a DRAM bounce to keep the SBUF write partition-outer
            attn_dram = dram.tile([1, tp], BF16, tag="attn_dram")
            nc.sync.dma_start(attn_dram, exp_bf)
            attn_t = small.tile([P, tc_n], BF16, tag="attn")
            nc.sync.dma_start(
                attn_t, attn_dram.rearrange("one (c p) -> p (one c)", p=P)
            )
            carry[b] = {"attn_t": attn_t, "vals": vals, "rec": rec}

        emit_attn_values(B - 1)

    nc.compile()
    return nc


def _get_built(tp):
    if tp not in _built:
        _built[tp] = _build(tp)
    return _built[tp]


N_CORES = 8


def _prep(query, keys, values, mask, W1, b1, W2, b2, w_score):
    """Host-side: compaction + layout/dtype prep. Returns (tp, in_maps)."""
    query = np.ascontiguousarray(np.asarray(query, dtype=np.float32).reshape(64, M))
    keys = np.asarray(keys, dtype=np.float32)
    values = np.asarray(values, dtype=np.float32)
    mask = np.asarray(mask, dtype=np.float32).reshape(64, T)

    kept = [np.flatnonzero(mask[i] < 0.5) for i in range(64)]
    max_kept = max((len(k) for k in kept), default=T)
    tp = min(T, max(P, -(-max_kept // P) * P))

    # compacted, padded, transposed keys + values per batch
    keys_t = np.zeros((64, M, tp), dtype=BF16NP)
    vals_c = np.zeros((64, tp, D), dtype=BF16NP)
    mask_c = np.ones((64, tp), dtype=np.float32)
    for i in range(64):
        n = len(kept[i])
        keys_t[i, :, :n] = keys[i, kept[i], :].T.astype(BF16NP)
        vals_c[i, :n, :] = values[i, kept[i], :].astype(BF16NP)
        mask_c[i, :n] = 0.0

    W1 = np.asarray(W1, dtype=np.float32)
    w1qc = np.ascontiguousarray((W1[0:M] + W1[2 * M:3 * M]).astype(BF16NP))
    w1bc = np.ascontiguousarray((W1[M:2 * M] - W1[2 * M:3 * M]).astype(BF16NP))
    w1d = np.ascontiguousarray(W1[3 * M:4 * M].astype(BF16NP))
    shared = {
        "w1qc": w1qc, "w1bc": w1bc, "w1d": w1d,
        "w2": np.ascontiguousarray(np.asarray(W2, dtype=np.float32).astype(BF16NP)),
        "b1": np.ascontiguousarray(np.asarray(b1, dtype=np.float32)),
        "b2": np.ascontiguousarray(np.asarray(b2, dtype=np.float32)),
        "w_score": np.ascontiguousarray(np.asarray(w_score, dtype=np.float32)),
    }
    in_maps = []
    for c in range(N_CORES):
        sl = slice(c * B, (c + 1) * B)
        in_maps.append({
            "query": query[sl],
            "keys_t": np.ascontiguousarray(keys_t[sl]),
            "values": np.ascontiguousarray(vals_c[sl]),
            "mask": np.ascontiguousarray(mask_c[sl]),
            **shared,
        })
    return tp, in_maps


def make_in_maps(query, keys, values, mask, W1, b1, W2, b2, w_score, b_score=None):
    # b_score is ignored: softmax is shift-invariant.
    return _prep(query, keys, values, mask, W1, b1, W2, b2, w_score)


def gather_out(results):
    out = np.concatenate([results[c]["out"] for c in range(N_CORES)], axis=0)
    return out.reshape(64, 1, D).astype(np.float32)


def kernel(query, keys, values, mask, W1, b1, W2, b2, w_score, b_score):
    """Full-input entry point: shards over 8 NeuronCores, returns [64, 1, D]."""
    from concourse.bass_utils import run_bass_kernel_spmd

    tp, in_maps = _prep(query, keys, values, mask, W1, b1, W2, b2, w_score)
    nc = _get_built(tp)
    res = run_bass_kernel_spmd(nc, in_maps, core_ids=list(range(N_CORES)))
    return gather_out(res.results)


# revision 3
# speedup vs baseline: 2.5635x; 1.3207x over previous
"""DIN attention layer kernel for Trainium2 (8 NeuronCores, data-parallel over batch).

Reference computation (per batch b):
    att = [q, k, q-k, q*k]            # [T, 4M]
    h1  = relu(att @ W1 + b1)         # [T, D]
    h2  = relu(h1 @ W2 + b2)          # [T, D]
    s   = h2 @ w_score + b_score      # [T, 1]
    attn = softmax(s.T + mask * -1e9) # [1, T]
    out = attn @ values               # [1, D]

Key optimizations:
  * Data-parallel: 8 batches per core (B=64 over 8 cores).
  * Masked-token compaction on the host: tokens with mask=1 get attention
    weight exactly 0 (exp(-1e9) underflows to 0 in fp32, identical to the
    reference softmax), so they are dropped before the device kernel.
    Batches are sorted by kept-token count and assigned round-robin, so
    batch-slot s runs with the exact max kept count of its rank group
    (degrades gracefully to dense T=1024 for any mask density).
  * Algebraic reassociation of the concat matmul:
        att @ W1 = q@(W1a+W1c) + k@(W1b-W1c) + (q*k)@W1d
    The q term is a per-batch row vector folded into the layer-1 bias;
    the q*k term is folded into the weights per batch:
        (q*k) @ W1d = k @ (diag(q) W1d)
    so layer 1 contracts over just M=256 (vs 1024 naively).
  * All tensors host-packed partition-major (one contiguous row per SBUF
    partition) and pre-cast to bf16 where applicable: minimal DMA
    descriptor counts, no device-side transposes or casts.
  * DMA issue spread over the three DGE queues (Sync, ACT, GpSimd).
  * Scores computed transposed ([token%128, tokchunk] layout) by using h2
    as the matmul stationary operand: softmax becomes small [128, tc]
    ops, attn lands in lhsT layout directly — no DRAM round-trip.
  * bf16 matmuls (fp32 PSUM accumulation); softmax in fp32.
  * Layer-1 PSUM drains alternate DVE/ACT so neither engine gates the PE.
  * attn@values for batch b is emitted inside batch b+1's mm2 loop so the
    PE never waits on the softmax chain.
"""

import numpy as np
import ml_dtypes

P = 128
B = 8          # batches per core
T = 1024       # tokens (full)
M = 256        # key feature dim
D = 1024       # hidden dim
MC = M // P    # key-feature chunks (2)
DC = D // P    # hidden chunks (8)
NEG = -1.0e9

BF16NP = ml_dtypes.bfloat16

_built = {}


def _chunks(tp):
    """Token free-dim chunks of width <= 512."""
    out = []
    off = 0
    while off < tp:
        w = min(512, tp - off)
        out.append((off, w))
        off += w
    return out


def _build(tps):
    import concourse.bass as bass  # noqa: F401
    import concourse.bacc as bacc
    import concourse.mybir as mybir
    import concourse.tile as tile
    from contextlib import ExitStack

    F32 = mybir.dt.float32
    BF16 = mybir.dt.bfloat16
    AF = mybir.ActivationFunctionType
    OP = mybir.AluOpType

    tcs = [-(-tp // P) for tp in tps]      # per-slot 128-token chunk counts
    tc_max = max(tcs)
    tp_pad = tc_max * P                    # padded token capacity (layouts)

    nc = bacc.Bacc("TRN2")
    # host-packed inputs: leading dim = SBUF partition, rows contiguous
    qtf_d = nc.dram_tensor("qt_f", [P, MC * B], F32, kind="ExternalInput").ap()
    qtb_d = nc.dram_tensor("qt_b", [P, MC * B], BF16, kind="ExternalInput").ap()
    b1_d = nc.dram_tensor("b1s", [P, DC], F32, kind="ExternalInput").ap()
    b2_d = nc.dram_tensor("b2s", [P, DC], F32, kind="ExternalInput").ap()
    ws_d = nc.dram_tensor("wss", [P, DC], BF16, kind="ExternalInput").ap()
    w1qc_d = nc.dram_tensor("w1qc", [P, MC * D], BF16, kind="ExternalInput").ap()
    w1bc_d = nc.dram_tensor("w1bc", [P, MC * D], BF16, kind="ExternalInput").ap()
    w1d_d = nc.dram_tensor("w1d", [P, MC * D], BF16, kind="ExternalInput").ap()
    w2_d = nc.dram_tensor("w2", [P, DC * D], BF16, kind="ExternalInput").ap()
    kt_d = nc.dram_tensor("keys_t", [B, P, MC * tp_pad], BF16,
                          kind="ExternalInput").ap()
    v_d = nc.dram_tensor("values", [B, P, tc_max * D], BF16,
                         kind="ExternalInput").ap()
    m_d = nc.dram_tensor("mask_t", [B, P, tc_max], F32, kind="ExternalInput").ap()
    out_d = nc.dram_tensor("out", [B, D], F32, kind="ExternalOutput").ap()

    with tile.TileContext(nc) as tc, ExitStack() as ctx:
        cons = ctx.enter_context(tc.tile_pool(name="cons", bufs=1))
        xpool = ctx.enter_context(tc.tile_pool(name="xpool", bufs=2))
        wepool = ctx.enter_context(tc.tile_pool(name="wep", bufs=2))
        vpool = ctx.enter_context(tc.tile_pool(name="vp", bufs=2))
        small = ctx.enter_context(tc.tile_pool(name="small", bufs=2))
        psum_mm = ctx.enter_context(tc.tile_pool(name="psmm", bufs=4, space="PSUM"))
        psum_sc = ctx.enter_context(tc.tile_pool(name="pssc", bufs=2, space="PSUM"))
        psum_vec = ctx.enter_context(tc.tile_pool(name="psvec", bufs=2, space="PSUM"))

        # ---- one-time setup -------------------------------------------------
        # critical path first: qt + w1bc/w1d (w1eff deps) on the Sync HWDGE
        # queue; w1qc/w2 on the ACT HWDGE queue; the rest on GpSimd SWDGE.
        qt_f = cons.tile([P, MC, B], F32)
        qt_b = cons.tile([P, MC, B], BF16)
        nc.sync.dma_start(qt_f, qtf_d.rearrange("p (c b) -> p c b", c=MC))
        nc.sync.dma_start(qt_b, qtb_d.rearrange("p (c b) -> p c b", c=MC))
        b1_sb = cons.tile([P, DC], F32)
        nc.sync.dma_start(b1_sb, b1_d)
        w1bc = cons.tile([P, MC, D], BF16)
        w1d_sb = cons.tile([P, MC, D], BF16)
        nc.sync.dma_start(w1bc, w1bc_d.rearrange("p (c d) -> p c d", c=MC))
        nc.sync.dma_start(w1d_sb, w1d_d.rearrange("p (c d) -> p c d", c=MC))
        w1qc = cons.tile([P, MC, D], BF16)
        nc.scalar.dma_start(w1qc, w1qc_d.rearrange("p (c d) -> p c d", c=MC))
        w2_sb = cons.tile([P, DC, D], BF16)
        nc.scalar.dma_start(w2_sb, w2_d.rearrange("p (c d) -> p c d", c=DC))
        b2_sb = cons.tile([P, DC], F32)
        ws_sb = cons.tile([P, DC], BF16)
        nc.gpsimd.dma_start(b2_sb, b2_d)
        nc.gpsimd.dma_start(ws_sb, ws_d)

        ones_bf = cons.tile([P, 1], BF16)
        nc.vector.memset(ones_bf, 1.0)

        # fixed activations buffers (reused across batches); h2 tails beyond
        # each slot's exact token count are read by the transposed score
        # matmuls — zero them once so they are always finite (masked later)
        h1buf = cons.tile([P, DC, tp_pad], BF16)
        h2buf = [cons.tile([P, tp_pad], BF16, name=f"h2_{i}") for i in range(2)]
        for i in range(2):
            nc.vector.memset(h2buf[i], 0.0)

        # RT[p, b, j] = (q_b @ (W1a+W1c))[j*128+p] + b1[j*128+p], on the PE
        rt = cons.tile([P, B, DC], F32)
        for j in range(DC):
            rt_ps = psum_vec.tile([P, B], F32, tag="vec", name=f"rt_ps{j}")
            for c in range(MC):
                nc.tensor.matmul(
                    rt_ps, w1qc[:, c, j * P:(j + 1) * P], qt_b[:, c, :],
                    start=(c == 0), stop=(c == MC - 1),
                )
            nc.vector.tensor_scalar(
                rt[:, :, j], rt_ps, b1_sb[:, j:j + 1], None, op0=OP.add,
            )

        # ---- per-batch pipeline --------------------------------------------
        carry = {}

        def emit_attn_values(b):
            st = carry.pop(b)
            tcn = st["tcn"]
            out_ps = [psum_vec.tile([1, 512], F32, tag="vec", name=f"o_ps{h}")
                      for h in range(2)]
            for h in range(2):
                for c in range(tcn):
                    nc.tensor.matmul(
                        out_ps[h],
                        st["attn_t"][:, c:c + 1],
                        st["vals"][:, c, h * 512:(h + 1) * 512],
                        start=(c == 0), stop=(c == tcn - 1),
                    )
            out_sb = small.tile([1, D], F32, tag="osb")
            for h in range(2):
                nc.vector.tensor_scalar_mul(
                    out_sb[:, h * 512:(h + 1) * 512], out_ps[h], st["rec"])
            nc.gpsimd.dma_start(out_d[b:b + 1, :], out_sb)

        for b in range(B):
            tp = tps[b]
            tcn = tcs[b]
            cks = _chunks(tp)

            # per-batch effective layer-1 weights: W1eff = W1bc + q_b * W1d
            w1eff = wepool.tile([P, MC, D], BF16, tag="weff")
            for c in range(MC):
                nc.vector.scalar_tensor_tensor(
                    w1eff[:, c, :], in0=w1d_sb[:, c, :], scalar=qt_f[:, c, b:b + 1],
                    in1=w1bc[:, c, :], op0=OP.mult, op1=OP.add,
                )

            x_t = xpool.tile([P, MC, tp], BF16, tag="X")
            nc.sync.dma_start(
                x_t, kt_d[b].rearrange("p (c t) -> p c t", t=tp_pad)[:, :, 0:tp])
            mask_t = small.tile([P, tcn], F32, tag="mask")
            nc.gpsimd.dma_start(mask_t, m_d[b][:, 0:tcn])
            vals = vpool.tile([P, tcn, D], BF16, tag="vals")
            nc.scalar.dma_start(
                vals, v_d[b].rearrange("p (to d) -> p to d", d=D)[:, 0:tcn, :])

            # mm1: H1[d, t] = relu(W1eff.T @ X + rt_b)
            for j in range(DC):
                for off, w in cks:
                    ps = psum_mm.tile([P, w], F32, tag="mm", name=f"m1_{j}_{off}")
                    for c in range(MC):
                        nc.tensor.matmul(
                            ps, w1eff[:, c, j * P:(j + 1) * P],
                            x_t[:, c, off:off + w],
                            start=(c == 0), stop=(c == MC - 1),
                        )
                    if j % 2 == 0:
                        nc.vector.tensor_scalar(
                            h1buf[:, j, off:off + w], ps, rt[:, b, j:j + 1], 0.0,
                            op0=OP.add, op1=OP.max,
                        )
                    else:
                        nc.scalar.activation(
                            h1buf[:, j, off:off + w], ps, AF.Relu,
                            bias=rt[:, b, j:j + 1], scale=1.0,
                        )

            # mm2 + transposed score: score_t[p, c2] = sum_d ws[d]*h2[d, tok]
            # with tok = c2*128 + p, via h2-chunk-stationary matmuls
            # (deferred by one j so the PE doesn't wait on the relu)
            score_ps = psum_sc.tile([P, tcn], F32, tag="sc")

            def emit_score(jj):
                h2p = h2buf[jj % 2]
                for c2 in range(tcn):
                    nc.tensor.matmul(
                        score_ps[:, c2:c2 + 1],
                        h2p[:, c2 * P:(c2 + 1) * P], ws_sb[:, jj:jj + 1],
                        start=(jj == 0), stop=(jj == DC - 1),
                        skip_group_check=True,
                    )

            for j in range(DC):
                h2 = h2buf[j % 2]
                for off, w in cks:
                    ps = psum_mm.tile([P, w], F32, tag="mm", name=f"m2_{j}_{off}")
                    for c in range(DC):
                        nc.tensor.matmul(
                            ps, w2_sb[:, c, j * P:(j + 1) * P],
                            h1buf[:, c, off:off + w],
                            start=(c == 0), stop=(c == DC - 1),
                        )
                    nc.scalar.activation(
                        h2[:, off:off + w], ps, AF.Relu,
                        bias=b2_sb[:, j:j + 1], scale=1.0,
                    )
                if j > 0:
                    emit_score(j - 1)
                # deferred attn@values for the previous batch: emitted behind
                # mm1 + one mm2 j-round of PE work so its softmax chain is
                # fully hidden
                if j == 1 and b > 0:
                    emit_attn_values(b - 1)
            emit_score(DC - 1)

            # softmax, transposed: attn_t[p, c2] = exp(score + mask*-1e9);
            # masked/pad lanes underflow to exactly 0 (scores are O(1))
            score_m = small.tile([P, tcn], F32, tag="ssb")
            nc.vector.scalar_tensor_tensor(
                score_m, in0=mask_t, scalar=NEG, in1=score_ps,
                op0=OP.mult, op1=OP.add,
            )
            partial = small.tile([P, 1], F32, tag="part")
            attn_t = small.tile([P, tcn], BF16, tag="attn")
            nc.scalar.activation(attn_t, score_m, AF.Exp, accum_out=partial)
            partial_bf = small.tile([P, 1], BF16, tag="partb")
            nc.vector.tensor_copy(partial_bf, partial)
            sum_ps = psum_sc.tile([1, 1], F32, tag="sc", name="sum_ps")
            nc.tensor.matmul(sum_ps, ones_bf, partial_bf, start=True, stop=True)
            rec = small.tile([1, 1], F32, tag="rec")
            nc.vector.reciprocal(rec, sum_ps)

            carry[b] = {"attn_t": attn_t, "vals": vals, "rec": rec, "tcn": tcn}

        emit_attn_values(B - 1)

    nc.compile()
    return nc


def _get_built(tps):
    key = tuple(tps)
    if key not in _built:
        _built[key] = _build(key)
    return _built[key]


N_CORES = 8


def _pack_rows(a, c):
    """[c*P, N] -> [P, c*N] with row p = concat_c a[c*P + p, :]."""
    n = a.shape[1]
    return np.ascontiguousarray(
        a.reshape(c, P, n).transpose(1, 0, 2).reshape(P, c * n))


def _prep(query, keys, values, mask, W1, b1, W2, b2, w_score):
    """Host-side: compaction + sorted slot assignment + layout/dtype prep."""
    query = np.asarray(query, dtype=np.float32).reshape(64, M)
    keys = np.asarray(keys, dtype=np.float32)
    values = np.asarray(values, dtype=np.float32)
    mask = np.asarray(mask, dtype=np.float32).reshape(64, T)

    kept = [np.flatnonzero(mask[i] < 0.5) for i in range(64)]
    # sort batches by kept count desc; slot s of core c <- rank (s*8 + c)
    order = np.argsort([-len(k) for k in kept], kind="stable")
    tps, tcs = [], []
    for s in range(B):
        grp = order[s * N_CORES:(s + 1) * N_CORES]
        tp = min(T, max(1, max(len(kept[g]) for g in grp)))
        tcn = -(-tp // P)
        tps.append(tp)
        tcs.append(tcn)
    tc_max = max(tcs)
    tp_pad = tc_max * P

    keys_t = np.zeros((64, P, MC * tp_pad), dtype=BF16NP)
    vals_c = np.zeros((64, P, tc_max * D), dtype=BF16NP)
    mask_c = np.ones((64, P, tc_max), dtype=np.float32)
    qt = np.zeros((64, M), dtype=np.float32)
    slot_of = {}
    for s in range(B):
        for c in range(N_CORES):
            g = order[s * N_CORES + c]
            slot_of[(c, s)] = g
            idx = kept[g]
            n = len(idx)
            i = c * B + s  # row in the packed per-core arrays
            kT = np.zeros((M, tp_pad), dtype=np.float32)
            kT[:, :n] = keys[g, idx, :].T
            keys_t[i] = _pack_rows(kT, MC).astype(BF16NP)
            v = np.zeros((tc_max * P, D), dtype=np.float32)
            v[:n] = values[g, idx, :]
            vals_c[i] = _pack_rows(v, tc_max).astype(BF16NP)
            mc = np.ones(tc_max * P, dtype=np.float32)
            mc[:n] = 0.0
            mask_c[i] = mc.reshape(tc_max, P).T
            qt[i] = query[g]

    W1 = np.asarray(W1, dtype=np.float32)
    w1qc = _pack_rows(W1[0:M] + W1[2 * M:3 * M], MC).astype(BF16NP)
    w1bc = _pack_rows(W1[M:2 * M] - W1[2 * M:3 * M], MC).astype(BF16NP)
    w1d = _pack_rows(W1[3 * M:4 * M], MC).astype(BF16NP)
    w2p = _pack_rows(np.asarray(W2, dtype=np.float32), DC).astype(BF16NP)

    def stripe(v):
        return np.ascontiguousarray(
            np.asarray(v, dtype=np.float32).reshape(-1)[: D].reshape(DC, P).T)

    b1s = stripe(b1)
    b2s = stripe(b2)
    wss = stripe(w_score).astype(BF16NP)

    shared = {
        "w1qc": w1qc, "w1bc": w1bc, "w1d": w1d, "w2": w2p,
        "b1s": b1s, "b2s": b2s, "wss": wss,
    }
    in_maps = []
    for c in range(N_CORES):
        rows = [c * B + s for s in range(B)]
        q_core = qt[rows]  # [B, M]
        qt_f = np.ascontiguousarray(
            q_core.reshape(B, MC, P).transpose(2, 1, 0).reshape(P, MC * B))
        in_maps.append({
            "qt_f": qt_f,
            "qt_b": qt_f.astype(BF16NP),
            "keys_t": np.ascontiguousarray(keys_t[rows]),
            "values": np.ascontiguousarray(vals_c[rows]),
            "mask_t": np.ascontiguousarray(mask_c[rows]),
            **shared,
        })
    return tps, slot_of, in_maps


def make_in_maps(query, keys, values, mask, W1, b1, W2, b2, w_score, b_score=None):
    # b_score is ignored: softmax is shift-invariant.
    return _prep(query, keys, values, mask, W1, b1, W2, b2, w_score)


def gather_out(results, slot_of):
    out = np.empty((64, 1, D), dtype=np.float32)
    for c in range(N_CORES):
        for s in range(B):
            out[slot_of[(c, s)], 0, :] = results[c]["out"][s]
    return out


def kernel(query, keys, values, mask, W1, b1, W2, b2, w_score, b_score):
    """Full-input entry point: shards over 8 NeuronCores, returns [64, 1, D]."""
    from concourse.bass_utils import run_bass_kernel_spmd

    tps, slot_of, in_maps = _prep(
        query, keys, values, mask, W1, b1, W2, b2, w_score)
    nc = _get_built(tps)
    res = run_bass_kernel_spmd(nc, in_maps, core_ids=list(range(N_CORES)))
    return gather_out(res.results, slot_of)
